# revision 1
# baseline (speedup 1.0000x reference)
"""AngleFusion kernel — data-parallel over batch B across 8 trn2 NeuronCores.

Full inputs in, full output out. Batch B=32 is split 4-per-core across the
8 cores; all params (<10MB) are replicated. The per-(batch,channel,head)
bmm-softmax-bmm chain is embarrassingly parallel along B, so each core
computes its 4 batches end-to-end with no cross-core communication; the
host gathers the 8 shards back into the full [32, 512, 32, 32] output.
"""

import numpy as np

B, C, H, W, NH = 32, 512, 32, 32, 2
LEN = H * W  # 1024
NCORES = 8
BS = B // NCORES  # 4 batches per core


def _compute_jnp(jnp, jnn, featuremap, angle, w1, b1, w2, b2, w3, b3,
                 wmh, bmh, conv_w, conv_b, gamma):
    """The fused math for one batch shard [BS, C, H, W]."""
    b = featuremap.shape[0]
    # ag2vec MLP on angle: [b,1] -> [b, h*w]
    av = jnn.relu(angle @ w1 + b1)
    av = jnn.relu(av @ w2 + b2)
    av = jnn.relu(av @ w3 + b3)
    amap = av.reshape(b, W, H)  # [b, w, h]

    # to_mutiheads: [b*C, LEN] @ [LEN, LEN*NH] -> [b, C*NH, H, W]
    fm = (featuremap.reshape(b * C, LEN) @ wmh + bmh).reshape(b, C * NH, H, W)

    # bmm1 + softmax(dim=w) / sqrt(W)
    fus = jnp.einsum('bwh,bnhv->bnwv', amap, fm)
    fus = jnn.softmax(fus, axis=2) / jnp.sqrt(jnp.float32(W))

    # bmm2
    fusion = jnp.einsum('bnhw,bnwv->bnhv', fm, fus)

    # 1x1 conv over channel-head dim + gated residual
    out = jnp.einsum('bnhw,cn->bchw', fusion, conv_w) + conv_b[None, :, None, None]
    return featuremap + gamma * out


def _kernel_numpy(featuremap, angle, w1, b1, w2, b2, w3, b3,
                  wmh, bmh, conv_w, conv_b, gamma):
    """Pure-numpy fallback (host), exact same math."""
    f32 = np.float32
    av = np.maximum(angle @ w1 + b1, 0).astype(f32)
    av = np.maximum(av @ w2 + b2, 0).astype(f32)
    av = np.maximum(av @ w3 + b3, 0).astype(f32)
    amap = av.reshape(B, W, H)
    fm = (featuremap.reshape(B * C, LEN) @ wmh + bmh).reshape(B, C * NH, H, W)
    fus = np.einsum('bwh,bnhv->bnwv', amap, fm)
    m = fus.max(axis=2, keepdims=True)
    e = np.exp(fus - m)
    fus = (e / e.sum(axis=2, keepdims=True)) / np.sqrt(f32(W))
    fusion = np.einsum('bnhw,bnwv->bnhv', fm, fus)
    out = np.einsum('bnhw,cn->bchw', fusion, conv_w) + conv_b[None, :, None, None]
    return (featuremap + gamma * out).astype(f32)


def kernel(**inputs) -> np.ndarray:
    featuremap = np.ascontiguousarray(inputs["featuremap"], dtype=np.float32)
    angle = np.ascontiguousarray(inputs["angle"], dtype=np.float32)
    params = {k: np.ascontiguousarray(inputs[k], dtype=np.float32)
              for k in ("w1", "b1", "w2", "b2", "w3", "b3",
                        "wmh", "bmh", "conv_w", "conv_b", "gamma")}
    try:
        import jax
        import jax.numpy as jnp
        import jax.nn as jnn
        devs = jax.devices()
        if len(devs) < NCORES:
            raise RuntimeError(f"need {NCORES} devices, got {len(devs)}")

        # Shard batch across the 8 cores: [8, 4, C, H, W]
        fm_sh = featuremap.reshape(NCORES, BS, C, H, W)
        an_sh = angle.reshape(NCORES, BS, 1)

        def per_core(fm_i, an_i, *ps):
            return _compute_jnp(jnp, jnn, fm_i, an_i, *ps)

        pnames = ("w1", "b1", "w2", "b2", "w3", "b3",
                  "wmh", "bmh", "conv_w", "conv_b", "gamma")
        fn = jax.pmap(per_core,
                      in_axes=(0, 0) + (None,) * len(pnames),
                      devices=devs[:NCORES])
        out_sh = fn(fm_sh, an_sh, *[params[k] for k in pnames])
        out = np.asarray(out_sh).reshape(B, C, H, W).astype(np.float32)
        return out
    except Exception:
        return _kernel_numpy(featuremap, angle, **params)


if __name__ == "__main__":
    rng = np.random.default_rng(0)
    ins = {
        "featuremap": rng.standard_normal((B, C, H, W), dtype=np.float32),
        "angle": rng.random((B, 1), dtype=np.float32),
        "w1": rng.standard_normal((1, LEN // 4), dtype=np.float32),
        "b1": np.zeros((LEN // 4,), np.float32),
        "w2": rng.standard_normal((LEN // 4, LEN // 2), dtype=np.float32) * 0.06,
        "b2": np.zeros((LEN // 2,), np.float32),
        "w3": rng.standard_normal((LEN // 2, LEN), dtype=np.float32) * 0.04,
        "b3": np.zeros((LEN,), np.float32),
        "wmh": rng.standard_normal((LEN, LEN * NH), dtype=np.float32) * 0.03,
        "bmh": np.zeros((LEN * NH,), np.float32),
        "conv_w": rng.standard_normal((C, NH * C), dtype=np.float32) * 0.03,
        "conv_b": np.zeros((C,), np.float32),
        "gamma": rng.standard_normal((1,), np.float32) * 0.1,
    }
    o = kernel(**ins)
    print(o.shape, o.dtype)


# revision 4
# speedup vs baseline: 2.4360x; 2.4360x over previous
"""AngleFusion kernel — data-parallel over batch B across 8 trn2 NeuronCores.

Full inputs in, full output out. Batch B=32 is split 4-per-core across the
8 cores; all params (<10MB) are replicated. The per-(batch,channel,head)
bmm-softmax-bmm chain is embarrassingly parallel along B, so each core
computes its 4 batches end-to-end with no cross-core communication; the
host gathers the 8 shards back into the full [32, 512, 32, 32] output.
"""

import numpy as np

B, C, H, W, NH = 32, 512, 32, 32, 2
LEN = H * W  # 1024
NCORES = 8
BS = B // NCORES  # 4 batches per core


def _compute_jnp(jnp, jnn, featuremap, angle, w1, b1, w2, b2, w3, b3,
                 wmh, bmh, conv_w, conv_b, gamma):
    """The fused math for one batch shard [BS, C, H, W]."""
    b = featuremap.shape[0]
    # ag2vec MLP on angle: [b,1] -> [b, h*w]
    av = jnn.relu(angle @ w1 + b1)
    av = jnn.relu(av @ w2 + b2)
    av = jnn.relu(av @ w3 + b3)
    amap = av.reshape(b, W, H)  # [b, w, h]

    # to_mutiheads: [b*C, LEN] @ [LEN, LEN*NH] -> [b, C*NH, H, W]
    fm = (featuremap.reshape(b * C, LEN) @ wmh + bmh).reshape(b, C * NH, H, W)

    # bmm1 + softmax(dim=w) / sqrt(W)
    fus = jnp.einsum('bwh,bnhv->bnwv', amap, fm)
    fus = jnn.softmax(fus, axis=2) / jnp.sqrt(jnp.float32(W))

    # bmm2
    fusion = jnp.einsum('bnhw,bnwv->bnhv', fm, fus)

    # 1x1 conv over channel-head dim + gated residual
    out = jnp.einsum('bnhw,cn->bchw', fusion, conv_w) + conv_b[None, :, None, None]
    return featuremap + gamma * out


def _kernel_numpy(featuremap, angle, w1, b1, w2, b2, w3, b3,
                  wmh, bmh, conv_w, conv_b, gamma):
    """Pure-numpy fallback (host), exact same math."""
    f32 = np.float32
    av = np.maximum(angle @ w1 + b1, 0).astype(f32)
    av = np.maximum(av @ w2 + b2, 0).astype(f32)
    av = np.maximum(av @ w3 + b3, 0).astype(f32)
    amap = av.reshape(B, W, H)
    fm = (featuremap.reshape(B * C, LEN) @ wmh + bmh).reshape(B, C * NH, H, W)
    fus = np.einsum('bwh,bnhv->bnwv', amap, fm)
    m = fus.max(axis=2, keepdims=True)
    e = np.exp(fus - m)
    fus = (e / e.sum(axis=2, keepdims=True)) / np.sqrt(f32(W))
    fusion = np.einsum('bnhw,bnwv->bnhv', fm, fus)
    out = np.einsum('bnhw,cn->bchw', fusion, conv_w) + conv_b[None, :, None, None]
    return (featuremap + gamma * out).astype(f32)


_PNAMES = ("w1", "b1", "w2", "b2", "w3", "b3",
           "wmh", "bmh", "conv_w", "conv_b", "gamma")
_CACHE: dict = {}


def _get_compiled(params):
    """Compile the per-core pmap fn once and pre-replicate params on-device."""
    key = hash(tuple(params[k].tobytes() for k in _PNAMES))
    if _CACHE.get("key") == key:
        return _CACHE["fn"], _CACHE["dev_params"]
    import jax
    import jax.numpy as jnp
    import jax.nn as jnn
    devs = jax.devices()
    if len(devs) < NCORES:
        raise RuntimeError(f"need {NCORES} devices, got {len(devs)}")
    devs = devs[:NCORES]

    def per_core(fm_i, an_i, *ps):
        return _compute_jnp(jnp, jnn, fm_i, an_i, *ps)

    fn = _CACHE.get("fn")
    if fn is None:
        fn = jax.pmap(per_core, in_axes=(0, 0) + (0,) * len(_PNAMES),
                      devices=devs)
    dev_params = [jax.device_put_replicated(params[k], devs) for k in _PNAMES]
    _CACHE["fn"] = fn
    _CACHE["dev_params"] = dev_params
    _CACHE["key"] = key
    return fn, dev_params


def kernel(**inputs) -> np.ndarray:
    featuremap = np.ascontiguousarray(inputs["featuremap"], dtype=np.float32)
    angle = np.ascontiguousarray(inputs["angle"], dtype=np.float32)
    params = {k: np.ascontiguousarray(inputs[k], dtype=np.float32)
              for k in _PNAMES}
    try:
        fn, dev_params = _get_compiled(params)
        # Shard batch across the 8 cores: [8, 4, C, H, W]
        fm_sh = featuremap.reshape(NCORES, BS, C, H, W)
        an_sh = angle.reshape(NCORES, BS, 1)
        out_sh = fn(fm_sh, an_sh, *dev_params)
        out = np.asarray(out_sh).reshape(B, C, H, W).astype(np.float32)
        return out
    except Exception:
        return _kernel_numpy(featuremap, angle, **params)


if __name__ == "__main__":
    rng = np.random.default_rng(0)
    ins = {
        "featuremap": rng.standard_normal((B, C, H, W), dtype=np.float32),
        "angle": rng.random((B, 1), dtype=np.float32),
        "w1": rng.standard_normal((1, LEN // 4), dtype=np.float32),
        "b1": np.zeros((LEN // 4,), np.float32),
        "w2": rng.standard_normal((LEN // 4, LEN // 2), dtype=np.float32) * 0.06,
        "b2": np.zeros((LEN // 2,), np.float32),
        "w3": rng.standard_normal((LEN // 2, LEN), dtype=np.float32) * 0.04,
        "b3": np.zeros((LEN,), np.float32),
        "wmh": rng.standard_normal((LEN, LEN * NH), dtype=np.float32) * 0.03,
        "bmh": np.zeros((LEN * NH,), np.float32),
        "conv_w": rng.standard_normal((C, NH * C), dtype=np.float32) * 0.03,
        "conv_b": np.zeros((C,), np.float32),
        "gamma": rng.standard_normal((1,), np.float32) * 0.1,
    }
    o = kernel(**ins)
    print(o.shape, o.dtype)
